# revision 41
# baseline (speedup 1.0000x reference)
"""Bass/Trainium2 kernel for nn_DTM (dynamic topic model) on 8 NeuronCores.

Strategy:
  - vocab-shard the decoder (betas / word_dist / recon loss) across 8 cores
  - batch-shard the encoder, AllGather theta
  - host sorts the batch by time index so word_dist is a per-time-group
    matmul with contraction K=50 (instead of one-hot contraction T*K)
  - softmax over the sharded vocab via online-softmax merge: one small
    AllGather of per-core (rowmax, rowsumexp), exp is computed once with
    the local max and the correction factor exp(m_loc-M)/S is folded into
    the theta-side matmul operand
  - losses fused on-chip; one final AllReduce of the partial sums
"""

import sys

if "/opt/trn_rl_repo" not in sys.path:
    sys.path.insert(0, "/opt/trn_rl_repo")

import numpy as np
import ml_dtypes

import concourse.bass as bass
import concourse.mybir as mybir
import concourse.tile as tile
from concourse import bacc
from concourse.bass_utils import run_bass_kernel_spmd
from concourse.masks import make_identity

F32 = mybir.dt.float32
F32R = mybir.dt.float32r
F16 = mybir.dt.float16
BF16 = mybir.dt.bfloat16
AX = mybir.AxisListType
ALU = mybir.AluOpType
ACTF = mybir.ActivationFunctionType

N_CORES = 8
KP = 64  # topics padded per time slice (so T*KP is a multiple of 128)

FULL_CFG = dict(B=2048, V=50000, K=50, T=10, D=1024, H=512)
KL_W = 0.5
EVO_W = 0.1
EPS = 1e-10
NEG_INF = -3.0e38
# exp values are stored in f16: scale them by 2^14 so the f16 subnormal
# cliff (~6e-8) lands below the eps floor of log(word_dist + eps). The
# factor cancels: S is computed from the scaled sums, the theta-side
# operand is rescaled by 2^14, and Ln's input scale removes it.
EXP_SHIFT = 14.0 * float(np.log(2.0))   # ln(2^14)
# the per-row softmax scale exp(m_loc-M)/S is folded into the exp tile
# (rhs of the word_dist matmul), scaled to 2^15*exp(l-M)/S so the wd-
# relevant band sits in f16 normal range; Ln's input scale removes it.
RHS_SCALE = 2.0 ** 15
INV_RHS_SCALE = 2.0 ** -15


def _ceil_div(a, b):
    return (a + b - 1) // b


def padded_layout(counts, T):
    """64-aligned per-group offsets in the padded batch (so word_dist
    matmul PSUM writes land on legal base partitions 0/64), total padded
    to a multiple of 128."""
    pad_off = []
    off = 0
    for t in range(T):
        pad_off.append(off)
        off += _ceil_div(int(counts[t]), 64) * 64
    B_pad = _ceil_div(off, 128) * 128
    return pad_off, B_pad


def build_program(cfg, counts, n_cores=N_CORES, debug=False, phase_limit=3):
    """Build the SPMD bass program. `counts[t]` = number of batch rows in
    time group t of the (host-sorted) batch; identical on every core."""
    B, V, K, T, D, H = (cfg[k] for k in ("B", "V", "K", "T", "D", "H"))
    assert B % n_cores == 0 and V % n_cores == 0 and B % 128 == 0
    BS = B // n_cores          # encoder batch shard
    VS = V // n_cores          # vocab shard
    DC = D // 128              # contraction chunks for D
    HC = H // 128              # contraction chunks for H
    R = T * KP                 # padded topic rows (640)
    RC = R // 128              # row chunks (5)
    assert R % 128 == 0 and KP % 2 == 0 and 128 % KP == 0

    # balanced vocab slices: all even (fp32r ISA restriction) and >= 256
    # where possible (f32r runs 1 cyc/row only at moving size >= 256)
    assert VS % 2 == 0
    n_sl = _ceil_div(VS, 512)
    h2, r2 = (VS // 2) // n_sl, (VS // 2) % n_sl
    ns_list = []
    voff = 0
    for i in range(n_sl):
        ns = 2 * (h2 + (1 if i < r2 else 0))
        ns_list.append((voff, ns))
        voff += ns
    assert voff == VS

    # wide (2-PSUM-bank) slices for the word_dist/loss phase
    ws_list = []
    voff = 0
    while voff < VS:
        wn = min(1024, VS - voff)
        ws_list.append((voff, wn))
        voff += wn
    NW = len(ws_list)

    # word_dist: full 128-row tiles over the 64-padded sorted batch; each
    # tile is covered by <=2 group segments starting at partition 0 or 64.
    # Segments span the padded group extents so every PSUM partition is
    # written (theta pad columns are zero -> wd pad rows are zero).
    offs = np.concatenate([[0], np.cumsum(counts)]).astype(int)
    pad_off, B_pad = padded_layout(counts, T)
    NT = B_pad // 128
    ends = [pad_off[t] + _ceil_div(int(counts[t]), 64) * 64 for t in range(T)]
    last_ne = max(t for t in range(T) if counts[t] > 0)
    ends[last_ne] = max(ends[last_ne], B_pad)
    segs_per_tile = []
    for i in range(NT):
        start, end = i * 128, (i + 1) * 128
        segs = []
        for t in range(T):
            if counts[t] == 0:
                continue
            lo, hi = max(start, pad_off[t]), min(end, ends[t])
            if hi > lo:
                segs.append((t, lo, lo - start, hi - lo))
        segs_per_tile.append(segs)
    NCOL = 2 * NT + 1  # S cols, denom cols, kl col

    evo_cols = (T - 1) * KP

    nc = bacc.Bacc("TRN2", target_bir_lowering=False, debug=debug,
                   num_devices=n_cores)

    # ---------------- I/O ----------------
    d_docT = nc.dram_tensor("docT", [D, BS], F32, kind="ExternalInput")
    d_mask = nc.dram_tensor("maskS", [T, BS], F32, kind="ExternalInput")
    d_W1 = nc.dram_tensor("W1", [D, H], F32, kind="ExternalInput")
    d_b1 = nc.dram_tensor("b1", [H, 1], F32, kind="ExternalInput")
    d_W2 = nc.dram_tensor("W2", [H, H], F32, kind="ExternalInput")
    d_b2 = nc.dram_tensor("b2", [H, 1], F32, kind="ExternalInput")
    d_WfT = nc.dram_tensor("WfT", [H, H], F32, kind="ExternalInput")
    d_WfB = nc.dram_tensor("WfB", [H, H], F32, kind="ExternalInput")
    d_bf = nc.dram_tensor("bf", [H, 1], F32, kind="ExternalInput")
    d_Wmu = nc.dram_tensor("Wmu", [H, K], F32, kind="ExternalInput")
    d_bmu = nc.dram_tensor("bmu", [K, 1], F32, kind="ExternalInput")
    d_Wlv = nc.dram_tensor("Wlv", [H, K], F32, kind="ExternalInput")
    d_blv = nc.dram_tensor("blv", [K, 1], F32, kind="ExternalInput")
    d_tableT = nc.dram_tensor("tableT", [H, T], F32, kind="ExternalInput")
    d_topicT = nc.dram_tensor("topicT", [D, R], F32, kind="ExternalInput")
    d_wordT = nc.dram_tensor("wordT", [D, VS], F32, kind="ExternalInput")
    d_bow = nc.dram_tensor("bow", [B_pad, VS], BF16, kind="ExternalInput")
    d_finsc = nc.dram_tensor("finsc", [4, 1], F32, kind="ExternalInput")
    d_finw = nc.dram_tensor("finw", [4, 1], F32, kind="ExternalInput")

    y_loss = nc.dram_tensor("loss", [1, 1], F32, kind="ExternalOutput")
    y_recon = nc.dram_tensor("recon", [1, 1], F32, kind="ExternalOutput")
    y_kl = nc.dram_tensor("kl", [1, 1], F32, kind="ExternalOutput")
    y_evo = nc.dram_tensor("evo", [1, 1], F32, kind="ExternalOutput")
    y_theta = nc.dram_tensor("theta", [B, K], F32, kind="ExternalOutput")

    rg = [list(range(n_cores))]

    with tile.TileContext(nc) as tc:
        with tc.tile_pool(name="persist", bufs=1) as pp, \
             tc.tile_pool(name="dram", bufs=1, space="DRAM") as dp:

            # ---------- constants ----------
            ident = pp.tile([128, 128], F32)
            make_identity(nc, ident)
            ones128 = pp.tile([128, 1], F32)
            nc.vector.memset(ones128, 1.0)
            klcol = pp.tile([128, 4], F32)
            nc.vector.memset(klcol, 0.0)
            accum_sb = pp.tile([128, NCOL], F32)
            nc.vector.memset(accum_sb, 0.0)
            eps_col = pp.tile([128, 1], F32)
            nc.vector.memset(eps_col, EPS)
            # per-slice partial accumulators (no cross-op chaining: the
            # chained TensorTensorReduce path is broken on this runtime)
            sc_a = pp.tile([128, NT * NW], F32)
            dc_a = pp.tile([128, NT * NW], F32)

            # ---------- persistent loads ----------
            topicT_sb = []
            for kc in range(DC):
                tpc = pp.tile([128, R], F32R, name=f"tpc{kc}")
                nc.sync.dma_start(
                    tpc[:, :],
                    d_topicT[kc * 128:(kc + 1) * 128, :].bitcast(F32R))
                topicT_sb.append(tpc)

            # theta^T replicated at partition offsets 0 and KP, so that the
            # word_dist matmul operands share a base partition for any t;
            # columns live at the 64-padded offsets, pad columns are zero
            thetaT_g = pp.tile([128, B_pad], F16)

            # logits / exp tiles (f16), 5 x [128, VS]
            lgx = []
            for m in range(RC):
                lt = pp.tile([128, VS], F16, name=f"lgx{m}")
                lgx.append(lt)
            mlt = pp.tile([128, RC], F32)      # local row max per chunk
            mxc = pp.tile([128, RC * n_sl], F32)  # per-slice row maxes
            slo = pp.tile([128, RC], F32)      # local row sumexp per chunk
            negm = pp.tile([128, RC], F32)
            sct = pp.tile([128, RC], F32)      # final scale per row chunk

            # collective bounce buffers
            ag2_in = dp.tile([BS, K], F32)
            ag2_out = dp.tile([B, K], F32)
            ag1_in = dp.tile([2, R], F32)
            ag1_out = dp.tile([2 * n_cores, R], F32)
            ar3_in = dp.tile([128, NCOL], F32)
            ar3_out = dp.tile([128, NCOL], F32)

            # ================= encoder (batch shard) =================
            with tc.tile_pool(name="enc", bufs=1) as ep, \
                 tc.tile_pool(name="eps", bufs=1, space="PSUM") as eps, \
                 tc.tile_pool(name="trp", bufs=2, space="PSUM") as trp:

                doc_sb = []
                for kc in range(DC):
                    dt_ = ep.tile([128, BS], F32R, name=f"doc{kc}")
                    nc.sync.dma_start(
                        dt_[:, :],
                        d_docT[kc * 128:(kc + 1) * 128, :].bitcast(F32R))
                    doc_sb.append(dt_)

                def load_w(dram, nch, width, nm, dt=F32):
                    out = []
                    for kc in range(nch):
                        w = ep.tile([128, width], dt, name=f"{nm}{kc}")
                        src_ = dram[kc * 128:(kc + 1) * 128, :]
                        if dt is F32R:
                            src_ = src_.bitcast(F32R)
                        nc.sync.dma_start(w[:, :], src_)
                        out.append(w)
                    return out

                w1_sb = load_w(d_W1, DC, H, "w1", F32R)
                w2_sb = load_w(d_W2, HC, H, "w2", F32R)
                wft_sb = load_w(d_WfT, HC, H, "wft", F32R)
                wfb_sb = load_w(d_WfB, HC, H, "wfb")
                wmu_sb = load_w(d_Wmu, HC, K, "wmu", F32R)
                wlv_sb = load_w(d_Wlv, HC, K, "wlv", F32R)
                tt_sb = load_w(d_tableT, HC, T, "tt")

                def load_bias(dram, nch, nm):
                    out = []
                    for m in range(nch):
                        b = ep.tile([128, 1], F32, name=f"{nm}{m}")
                        nc.sync.dma_start(b[:, :], dram[m * 128:(m + 1) * 128, :])
                        out.append(b)
                    return out

                b1_sb = load_bias(d_b1, HC, "b1s")
                b2_sb = load_bias(d_b2, HC, "b2s")
                bf_sb = load_bias(d_bf, HC, "bfs")
                bmu_sb = ep.tile([K, 1], F32)
                nc.sync.dma_start(bmu_sb[:, :], d_bmu[:, :])
                blv_sb = ep.tile([K, 1], F32)
                nc.sync.dma_start(blv_sb[:, :], d_blv[:, :])
                mask_sb = ep.tile([T, BS], F32)
                nc.sync.dma_start(mask_sb[:, :], d_mask[:, :])

                # G = time_emb_table @ Wf_bot   -> [T, H]
                g_psum = eps.tile([T, H], F32, tag="eps")
                for kc in range(HC):
                    nc.tensor.matmul(g_psum[:, :], tt_sb[kc][:, :], wfb_sb[kc][:, :],
                                     start=(kc == 0), stop=(kc == HC - 1))
                g_sb = ep.tile([T, H], F32)
                nc.vector.tensor_copy(g_sb[:, :], g_psum[:, :])

                # h1T = relu(W1.T @ docT + b1)
                h1_sb = [ep.tile([128, BS], F32R, name=f"h1_{m}") for m in range(HC)]
                for m in range(HC):
                    ps = eps.tile([128, BS], F32, tag="eps", name=f"ps1_{m}")
                    for kc in range(DC):
                        nc.tensor.matmul(
                            ps[:, :],
                            w1_sb[kc][:, m * 128:(m + 1) * 128],
                            doc_sb[kc][:, :],
                            start=(kc == 0), stop=(kc == DC - 1))
                    nc.scalar.activation(h1_sb[m][:, :], ps[:, :], ACTF.Relu,
                                         bias=b1_sb[m][:, 0:1], scale=1.0)

                # h2T = relu(W2.T @ h1T + b2)
                h2_sb = [ep.tile([128, BS], F32R, name=f"h2_{m}") for m in range(HC)]
                for m in range(HC):
                    ps = eps.tile([128, BS], F32, tag="eps", name=f"ps2_{m}")
                    for kc in range(HC):
                        nc.tensor.matmul(
                            ps[:, :],
                            w2_sb[kc][:, m * 128:(m + 1) * 128],
                            h1_sb[kc][:, :],
                            start=(kc == 0), stop=(kc == HC - 1))
                    nc.scalar.activation(h2_sb[m][:, :], ps[:, :], ACTF.Relu,
                                         bias=b2_sb[m][:, 0:1], scale=1.0)

                # hiddenT = relu(WfT.T @ h2T + G.T @ mask + bf)
                hid_sb = [ep.tile([128, BS], F32R, name=f"hd_{m}") for m in range(HC)]
                for m in range(HC):
                    ps = eps.tile([128, BS], F32, tag="eps", name=f"ps3_{m}")
                    for kc in range(HC):
                        nc.tensor.matmul(
                            ps[:, :],
                            wft_sb[kc][:, m * 128:(m + 1) * 128],
                            h2_sb[kc][:, :],
                            start=(kc == 0), stop=False)
                    nc.tensor.matmul(ps[:, :],
                                     g_sb[:, m * 128:(m + 1) * 128],
                                     mask_sb[:, :],
                                     start=False, stop=True)
                    nc.scalar.activation(hid_sb[m][:, :], ps[:, :], ACTF.Relu,
                                         bias=bf_sb[m][:, 0:1], scale=1.0)

                # muT / lvT  [K, BS]
                mu_sb = ep.tile([K, BS], F32)
                lv_sb = ep.tile([K, BS], F32)
                for dst, wsb, bsb in ((mu_sb, wmu_sb, bmu_sb),
                                      (lv_sb, wlv_sb, blv_sb)):
                    ps = eps.tile([K, BS], F32, tag="eps", name="psml")
                    for kc in range(HC):
                        nc.tensor.matmul(ps[:, :],
                                         wsb[kc][:, :],
                                         hid_sb[kc][:, :],
                                         start=(kc == 0), stop=(kc == HC - 1))
                    nc.scalar.activation(dst[:, :], ps[:, :], ACTF.Identity,
                                         bias=bsb[:, 0:1], scale=1.0)

                # theta (softmax over K) + KL partial, per 128-row batch tile
                for j in range(_ceil_div(BS, 128)):
                    pb = min(128, BS - j * 128)
                    cs = slice(j * 128, j * 128 + pb)
                    mu_ps = trp.tile([128, K], F32, tag="trp", name="mu_ps")
                    nc.tensor.matmul(mu_ps[0:pb, :], mu_sb[:, cs],
                                     ident[0:K, 0:K], start=True, stop=True)
                    lv_ps = trp.tile([128, K], F32, tag="trp", name="lv_ps")
                    nc.tensor.matmul(lv_ps[0:pb, :], lv_sb[:, cs],
                                     ident[0:K, 0:K], start=True, stop=True)
                    mu_bt = ep.tile([128, K], F32, tag="mu_bt", bufs=2)
                    lv_bt = ep.tile([128, K], F32, tag="lv_bt", bufs=2)
                    nc.vector.tensor_copy(mu_bt[0:pb, :], mu_ps[0:pb, :])
                    nc.vector.tensor_copy(lv_bt[0:pb, :], lv_ps[0:pb, :])

                    rmax = ep.tile([128, 1], F32, tag="rmax", bufs=2)
                    nc.vector.reduce_max(rmax[0:pb, :], mu_bt[0:pb, :], axis=AX.X)
                    nmx = ep.tile([128, 1], F32, tag="nmx", bufs=2)
                    nc.vector.tensor_scalar_mul(nmx[0:pb, :], rmax[0:pb, :], -1.0)
                    e_bt = ep.tile([128, K], F32, tag="e_bt", bufs=2)
                    sume = ep.tile([128, 1], F32, tag="sume", bufs=2)
                    nc.scalar.activation(e_bt[0:pb, :], mu_bt[0:pb, :], ACTF.Exp,
                                         bias=nmx[0:pb, 0:1], scale=1.0,
                                         accum_out=sume[0:pb, 0:1])
                    rcp = ep.tile([128, 1], F32, tag="rcp", bufs=2)
                    nc.vector.reciprocal(rcp[0:pb, :], sume[0:pb, :])
                    th_bt = ep.tile([128, K], F32, tag="th_bt", bufs=2)
                    nc.vector.tensor_scalar(th_bt[0:pb, :], e_bt[0:pb, :],
                                            rcp[0:pb, 0:1], None, op0=ALU.mult)
                    nc.sync.dma_start(ag2_in[cs, :], th_bt[0:pb, :])

                    # KL partial: sum(1 + lv - mu^2 - exp(lv))
                    mu2 = ep.tile([128, K], F32, tag="mu2", bufs=2)
                    nc.vector.tensor_mul(mu2[0:pb, :], mu_bt[0:pb, :], mu_bt[0:pb, :])
                    elv = ep.tile([128, K], F32, tag="elv", bufs=2)
                    nc.scalar.activation(elv[0:pb, :], lv_bt[0:pb, :], ACTF.Exp)
                    t1 = ep.tile([128, K], F32, tag="t1", bufs=2)
                    nc.vector.scalar_tensor_tensor(t1[0:pb, :], lv_bt[0:pb, :], 1.0,
                                                   mu2[0:pb, :], op0=ALU.add,
                                                   op1=ALU.subtract)
                    jk = ep.tile([128, K], F32, tag="jk", bufs=2)
                    nc.vector.scalar_tensor_tensor(
                        jk[0:pb, :], t1[0:pb, :], 0.0, elv[0:pb, :],
                        op0=ALU.add, op1=ALU.subtract,
                        accum_out=klcol[0:pb, j:j + 1])

                # theta AllGather
                nc.gpsimd.collective_compute(
                    "AllGather", ALU.bypass, replica_groups=rg,
                    ins=[ag2_in.opt()], outs=[ag2_out.opt()])
                nc.sync.dma_start(y_theta[:, :], ag2_out[:, :])

            if phase_limit < 2:
                nc.sync.dma_start(y_loss[:, :], klcol[0:1, 0:1])
                nc.sync.dma_start(y_recon[:, :], klcol[0:1, 0:1])
                nc.sync.dma_start(y_kl[:, :], klcol[0:1, 0:1])
                nc.sync.dma_start(y_evo[:, :], klcol[0:1, 0:1])

            # ================= logits + softmax stats =================
            if phase_limit >= 2:
              with tc.tile_pool(name="lg", bufs=1) as lp, \
                 tc.tile_pool(name="lgp", bufs=4, space="PSUM") as lgp:
                for ni, (voff, ns) in enumerate(ns_list):
                    wt = []
                    for kc in range(DC):
                        w = lp.tile([128, ns], F32R, tag=f"wt{kc}", bufs=2,
                                    name=f"wt{kc}_{ni}")
                        nc.sync.dma_start(
                            w[:, :],
                            d_wordT[kc * 128:(kc + 1) * 128,
                                    voff:voff + ns].bitcast(F32R))
                        wt.append(w)
                    for m in range(RC):
                        ps = lgp.tile([128, ns], F32, tag="lgps", bufs=3,
                                       name=f"lgps{m}_{ni}")
                        for kc in range(DC):
                            nc.tensor.matmul(
                                ps[:, :],
                                topicT_sb[kc][:, m * 128:(m + 1) * 128],
                                wt[kc][:, :],
                                start=(kc == 0), stop=(kc == DC - 1))
                        # drain psum: copy to f16 + row max in one DVE op
                        nc.vector.tensor_scalar(
                            lgx[m][:, voff:voff + ns], ps[:, :], 0.0, None,
                            op0=ALU.add, op1=ALU.max,
                            accum_out=mxc[:, m * n_sl + ni:m * n_sl + ni + 1])

                # thetaT from gathered theta (AG2 long done), rows
                # duplicated at offsets 0/KP, 64-padded column layout.
                # Emitted after the logits matmuls so the PE queue never
                # stalls on the AllGather.
                nc.vector.memset(thetaT_g, 0.0)
                for t in range(T):
                    cnt = int(counts[t])
                    for j in range(_ceil_div(cnt, 128)):
                        pb = min(128, cnt - j * 128)
                        srow = int(offs[t]) + j * 128
                        dcol = pad_off[t] + j * 128
                        th_g = lp.tile([128, K], F32, tag="th_g", bufs=3,
                                       name="th_g")
                        nc.sync.dma_start(th_g[0:pb, :],
                                          ag2_out[srow:srow + pb, :])
                        th_g2 = lp.tile([128, KP + K], F32, tag="th_g2",
                                        bufs=3, name="th_g2")
                        nc.vector.memset(th_g2[0:pb, K:KP], 0.0)
                        nc.vector.tensor_copy(th_g2[0:pb, 0:K], th_g[0:pb, :])
                        nc.vector.tensor_copy(th_g2[0:pb, KP:KP + K],
                                              th_g[0:pb, :])
                        tp = lgp.tile([KP + K, 128], F32, tag="tp", bufs=2,
                                      name="tp")
                        nc.tensor.matmul(tp[:, 0:pb], th_g2[0:pb, :],
                                         ident[0:pb, 0:pb], start=True,
                                         stop=True)
                        dc = slice(dcol, dcol + pb)
                        nc.vector.tensor_copy(thetaT_g[0:K, dc], tp[0:K, 0:pb])
                        nc.vector.tensor_copy(thetaT_g[KP:KP + K, dc],
                                              tp[KP:KP + K, 0:pb])

                # exp in place (f16), with local-max shift; accumulate row sums
                for m in range(RC):
                    nc.vector.reduce_max(mlt[:, m:m + 1],
                                         mxc[:, m * n_sl:(m + 1) * n_sl],
                                         axis=AX.X)
                    nc.vector.tensor_scalar(negm[:, m:m + 1], mlt[:, m:m + 1],
                                            -1.0, EXP_SHIFT,
                                            op0=ALU.mult, op1=ALU.add)
                    nc.scalar.activation(lgx[m][:, :], lgx[m][:, :], ACTF.Exp,
                                         bias=negm[:, m:m + 1], scale=1.0,
                                         accum_out=slo[:, m:m + 1])

                # ship (rowmax, rowsum) to every core
                for m in range(RC):
                    nc.sync.dma_start(ag1_in[0:1, m * 128:(m + 1) * 128],
                                      mlt[:, m:m + 1])
                    nc.sync.dma_start(ag1_in[1:2, m * 128:(m + 1) * 128],
                                      slo[:, m:m + 1])
                nc.gpsimd.collective_compute(
                    "AllGather", ALU.bypass, replica_groups=rg,
                    ins=[ag1_in.opt()], outs=[ag1_out.opt()])

                # global max M, global sum S, my scale = exp(m_loc - M)/S
                stats = ag1_out.rearrange("(i s) r -> s r i", s=2)
                for c in range(RC):
                    rs = slice(c * 128, (c + 1) * 128)
                    m8 = lp.tile([128, n_cores], F32, tag="m8", bufs=2, name="m8")
                    nc.sync.dma_start(m8[:, :], stats[0, rs, :])
                    s8 = lp.tile([128, n_cores], F32, tag="s8", bufs=2, name="s8")
                    nc.sync.dma_start(s8[:, :], stats[1, rs, :])
                    gmax = lp.tile([128, 1], F32, tag="gmax", bufs=2, name="gmax")
                    nc.vector.reduce_max(gmax[:, :], m8[:, :], axis=AX.X)
                    negM = lp.tile([128, 1], F32, tag="negM", bufs=2, name="negM")
                    nc.vector.tensor_scalar_mul(negM[:, :], gmax[:, :], -1.0)
                    d8 = lp.tile([128, n_cores], F32, tag="d8", bufs=2, name="d8")
                    nc.vector.tensor_scalar(d8[:, :], m8[:, :], negM[:, 0:1], None,
                                            op0=ALU.add)
                    e8 = lp.tile([128, n_cores], F32, tag="e8", bufs=2, name="e8")
                    nc.scalar.activation(e8[:, :], d8[:, :], ACTF.Exp)
                    w8 = lp.tile([128, n_cores], F32, tag="w8", bufs=2, name="w8")
                    Scol = lp.tile([128, 1], F32, tag="Scol", bufs=2, name="Scol")
                    nc.vector.scalar_tensor_tensor(
                        w8[:, :], e8[:, :], 0.0, s8[:, :],
                        op0=ALU.add, op1=ALU.mult, accum_out=Scol[:, 0:1])
                    rS = lp.tile([128, 1], F32, tag="rS", bufs=2, name="rS")
                    nc.vector.reciprocal(rS[:, :], Scol[:, :])
                    emy = lp.tile([128, 1], F32, tag="emy", bufs=2, name="emy")
                    nc.scalar.activation(emy[:, :], mlt[:, c:c + 1], ACTF.Exp,
                                         bias=negM[:, 0:1], scale=1.0)
                    # sct = 2^15 * exp(m_loc - M) / S'  (see RHS_SCALE note)
                    nc.vector.scalar_tensor_tensor(sct[:, c:c + 1], emy[:, :],
                                                   RHS_SCALE, rS[:, :],
                                                   op0=ALU.mult, op1=ALU.mult)
                    # fold the row scale into the exp tile (in place, f16)
                    nc.vector.tensor_scalar(lgx[c][:, :], lgx[c][:, :],
                                            sct[:, c:c + 1], None, op0=ALU.mult)

            if phase_limit == 2:
                nc.sync.dma_start(y_loss[:, :], sct[0:1, 0:1])
                nc.sync.dma_start(y_recon[:, :], sct[0:1, 0:1])
                nc.sync.dma_start(y_kl[:, :], klcol[0:1, 0:1])
                nc.sync.dma_start(y_evo[:, :], klcol[0:1, 0:1])

            # ================= word_dist + recon loss =================
            if phase_limit >= 3:
              with tc.tile_pool(name="wd", bufs=1) as wp, \
                 tc.tile_pool(name="wdp", bufs=3, space="PSUM") as wdp:
                for ti in range(NT):
                    segs = segs_per_tile[ti]
                    rbase = ti * 128
                    for wi, (voff, wn) in enumerate(ws_list):
                        bw = wp.tile([128, wn], BF16, tag="bw", bufs=4,
                                     name="bw")
                        nc.sync.dma_start(
                            bw[:, :],
                            d_bow[rbase:rbase + 128, voff:voff + wn])
                        ps = wdp.tile([128, wn], F32, tag="wdps", bufs=3,
                                      name="wdps")
                        for (t, gcol, lo, nrow) in segs:
                            c, po = t // 2, KP * (t % 2)
                            for sub in range(0, wn, 512):
                                sw = min(512, wn - sub)
                                nc.tensor.matmul(
                                    ps[lo:lo + nrow, sub:sub + sw],
                                    thetaT_g[po:po + K, gcol:gcol + nrow],
                                    lgx[c][po:po + K,
                                           voff + sub:voff + sub + sw],
                                    start=True, stop=True)
                        lnt = wp.tile([128, wn], BF16, tag="lnt", bufs=3,
                                      name="lnt")
                        nc.scalar.activation(lnt[:, :], ps[:, :], ACTF.Ln,
                                             bias=eps_col[:, 0:1],
                                             scale=INV_RHS_SCALE)
                        col = ti * NW + wi
                        prod = wp.tile([128, wn], BF16, tag="prod", bufs=2,
                                       name="prod")
                        nc.vector.tensor_mul(prod[:, :], bw[:, :], lnt[:, :])
                        scr = wp.tile([128, wn], BF16, tag="scr", bufs=2,
                                      name="scr")
                        nc.vector.tensor_scalar(
                            scr[:, :], prod[:, :], 0.0, None,
                            op0=ALU.add, op1=ALU.add,
                            accum_out=sc_a[:, col:col + 1])
                        scr2 = wp.tile([128, wn], BF16, tag="scr2", bufs=2,
                                       name="scr2")
                        nc.vector.tensor_scalar(
                            scr2[:, :], bw[:, :], 0.0, None,
                            op0=ALU.add, op1=ALU.add,
                            accum_out=dc_a[:, col:col + 1])

                for ti in range(NT):
                    nc.vector.reduce_sum(accum_sb[:, ti:ti + 1],
                                         sc_a[:, ti * NW:(ti + 1) * NW],
                                         axis=AX.X)
                    nc.vector.reduce_sum(accum_sb[:, NT + ti:NT + ti + 1],
                                         dc_a[:, ti * NW:(ti + 1) * NW],
                                         axis=AX.X)
                nc.vector.reduce_sum(accum_sb[:, 2 * NT:2 * NT + 1],
                                     klcol[:, :], axis=AX.X)
                nc.sync.dma_start(ar3_in[:, :], accum_sb[:, :])
                nc.gpsimd.collective_compute(
                    "AllReduce", ALU.add, replica_groups=rg,
                    ins=[ar3_in.opt()], outs=[ar3_out.opt()])

                # ---------------- final scalars ----------------
                acc2 = wp.tile([128, NCOL], F32)
                nc.sync.dma_start(acc2[:, :], ar3_out[:, :])
                finsc = wp.tile([4, 1], F32)
                nc.sync.dma_start(finsc[:, :], d_finsc[:, :])
                finw = wp.tile([4, 1], F32)
                nc.sync.dma_start(finw[:, :], d_finw[:, :])

                combo = wp.tile([128, 4], F32)
                nc.vector.memset(combo, 0.0)
                dn = wp.tile([128, NT], F32)
                nc.vector.tensor_scalar_add(dn[:, :], acc2[:, NT:2 * NT], EPS)
                rD = wp.tile([128, NT], F32)
                nc.vector.reciprocal(rD[:, :], dn[:, :])
                j3 = wp.tile([128, NT], F32)
                nc.vector.scalar_tensor_tensor(
                    j3[:, :], acc2[:, 0:NT], 0.0, rD[:, :],
                    op0=ALU.add, op1=ALU.mult, accum_out=combo[:, 0:1])
                nc.vector.tensor_copy(combo[:, 1:2], acc2[:, 2 * NT:2 * NT + 1])

                ev8 = wp.tile([128, DC], F32)
                for kc in range(DC):
                    dsub = wp.tile([128, evo_cols], F32, tag="dsub", bufs=2,
                                   name="dsub")
                    nc.vector.tensor_sub(
                        dsub[:, :],
                        topicT_sb[kc][:, KP:KP + evo_cols].bitcast(F32),
                        topicT_sb[kc][:, 0:evo_cols].bitcast(F32))
                    nc.scalar.activation(dsub[:, :], dsub[:, :], ACTF.Square,
                                         accum_out=ev8[:, kc:kc + 1])
                nc.vector.reduce_sum(combo[:, 2:3], ev8[:, :], axis=AX.X)

                with tc.tile_pool(name="fp", bufs=2, space="PSUM") as fp:
                    pf = fp.tile([4, 1], F32, tag="fin")
                    nc.tensor.matmul(pf[:, :], combo[:, :], ones128[:, :])
                    vals = wp.tile([4, 1], F32)
                    nc.vector.tensor_mul(vals[:, :], pf[:, :], finsc[:, :])
                    pl = fp.tile([1, 1], F32, tag="fin")
                    nc.tensor.matmul(pl[:, :], vals[:, :], finw[:, :])
                    loss_sb = wp.tile([1, 1], F32)
                    nc.vector.tensor_copy(loss_sb[:, :], pl[:, :])

                nc.sync.dma_start(y_recon[:, :], vals[0:1, 0:1])
                nc.sync.dma_start(y_kl[:, :], vals[1:2, 0:1])
                nc.sync.dma_start(y_evo[:, :], vals[2:3, 0:1])
                nc.sync.dma_start(y_loss[:, :], loss_sb[:, :])

    nc.compile()
    return nc


def prepare_inputs(cfg, inputs, n_cores=N_CORES):
    """Host-side staging: sort batch by time, shard, transpose. Returns
    (in_maps, perm, counts)."""
    B, V, K, T, D, H = (cfg[k] for k in ("B", "V", "K", "T", "D", "H"))
    BS = B // n_cores
    VS = V // n_cores
    R = T * KP

    t_idx = np.asarray(inputs["time_indices"]).astype(np.int64).ravel()
    perm = np.argsort(t_idx, kind="stable")
    t_sorted = t_idx[perm]
    counts = np.bincount(t_sorted, minlength=T).astype(np.int64)

    doc = np.asarray(inputs["doc_embeddings"], dtype=np.float32)
    bow = np.asarray(inputs["bow"], dtype=np.float32)
    docT = np.ascontiguousarray(doc[perm].T)                      # (D, B)
    mask_full = (np.arange(T)[:, None] == t_sorted[None, :]).astype(np.float32)

    # bow rows at the 64-padded group offsets (pad rows zero)
    pad_off, B_pad = padded_layout(counts, T)
    offs = np.concatenate([[0], np.cumsum(counts)]).astype(int)
    bow_perm = bow[perm]
    bow_s = np.zeros((B_pad, V), dtype=ml_dtypes.bfloat16)
    for t in range(T):
        cnt = int(counts[t])
        if cnt:
            bow_s[pad_off[t]:pad_off[t] + cnt] = \
                bow_perm[int(offs[t]):int(offs[t]) + cnt]

    we = np.asarray(inputs["word_embeddings"], dtype=np.float32)  # (V, D)
    te = np.asarray(inputs["topic_embeddings"], dtype=np.float32) # (T, K, D)
    topicT_pad = np.zeros((D, R), dtype=np.float32)
    for t in range(T):
        topicT_pad[:, t * KP:t * KP + K] = te[t].T

    Wf = np.asarray(inputs["Wf"], dtype=np.float32)
    finsc = np.array([[-1.0 / B], [-0.5 / B],
                      [1.0 / ((T - 1) * K * D)], [0.0]], dtype=np.float32)
    finw = np.array([[1.0], [KL_W], [EVO_W], [0.0]], dtype=np.float32)

    common = dict(
        W1=np.asarray(inputs["W1"], np.float32),
        b1=np.asarray(inputs["b1"], np.float32).reshape(H, 1),
        W2=np.asarray(inputs["W2"], np.float32),
        b2=np.asarray(inputs["b2"], np.float32).reshape(H, 1),
        WfT=np.ascontiguousarray(Wf[:H]),
        WfB=np.ascontiguousarray(Wf[H:]),
        bf=np.asarray(inputs["bf"], np.float32).reshape(H, 1),
        Wmu=np.asarray(inputs["Wmu"], np.float32),
        bmu=np.asarray(inputs["bmu"], np.float32).reshape(K, 1),
        Wlv=np.asarray(inputs["Wlv"], np.float32),
        blv=np.asarray(inputs["blv"], np.float32).reshape(K, 1),
        tableT=np.ascontiguousarray(
            np.asarray(inputs["time_emb_table"], np.float32).T),
        topicT=topicT_pad,
        finsc=finsc,
        finw=finw,
    )

    in_maps = []
    for r in range(n_cores):
        vs = slice(r * VS, (r + 1) * VS)
        bs = slice(r * BS, (r + 1) * BS)
        m = dict(common)
        m["docT"] = np.ascontiguousarray(docT[:, bs])
        m["maskS"] = np.ascontiguousarray(mask_full[:, bs])
        m["wordT"] = np.ascontiguousarray(we[vs].T)
        m["bow"] = np.ascontiguousarray(bow_s[:, vs])
        in_maps.append(m)
    return in_maps, perm, counts


def postprocess(results, perm, cfg):
    B, K = cfg["B"], cfg["K"]
    r0 = results[0]
    theta_sorted = np.asarray(r0["theta"], np.float32)
    theta = np.empty((B, K), np.float32)
    theta[perm] = theta_sorted
    loss = np.float32(r0["loss"].reshape(())[()])
    recon = np.float32(r0["recon"].reshape(())[()])
    kl = np.float32(r0["kl"].reshape(())[()])
    evo = np.float32(r0["evo"].reshape(())[()])
    return (np.asarray(loss), np.asarray(recon), np.asarray(kl),
            np.asarray(evo), theta)


def kernel(**inputs):
    cfg = FULL_CFG
    in_maps, perm, counts = prepare_inputs(cfg, inputs)
    nc = build_program(cfg, counts)
    res = run_bass_kernel_spmd(nc, in_maps, list(range(N_CORES)))
    return postprocess(res.results, perm, cfg)
